# revision 1
# baseline (speedup 1.0000x reference)
"""Bass/Trainium2 SPMD kernel for GQA causal attention with RoPE.

Sharding (8 cores): core c = 4*b + j (b = batch, j = 0..3 shard in batch).
  - Q / attention / o_proj: token-sharded; core j owns q-token 128-blocks
    {j, 7-j, j+8, 15-j} (balanced causal work, uniform SPMD program with
    fixed per-slot key extents [512, 1024, 1536, 2048] and data-driven masks).
  - K and V: token-sharded (core j computes tokens [512j, 512j+512), all
    dims; K roped in (t, d) layout). One fused AllGather per 4-core group
    assembles full K and V; K is PE-transposed on chip to (d, t) tiles.
  - Attention in s^T = (kpos, q) layout: scores = k^T.T @ q^T, exp on ACT
    (scale folds 1/sqrt(hd)), AV with ones-augmented V gives softmax sums,
    division applied after AV (commutes with the linear AV/o_proj steps).
Matmuls run in fp32r (full-rate fp32 on TRN2 at free-dim >= 256); the Q and
output projections use bf16 operands (halves their DMA traffic; verified
end-to-end relative error ~2.7e-3).
"""
import numpy as np

import concourse.bass as bass
import concourse.tile as tile
from concourse import bacc, mybir
from concourse.bass_utils import run_bass_kernel_spmd

B, T, D = 2, 2048, 2048
H, KV, HD = 32, 8, 64
P = 128
NB = T // P          # 16 token blocks of 128
OWN = 4 * P          # 512 owned q tokens per core
f32 = mybir.dt.float32
f32r = mybir.dt.float32r
bf16 = mybir.dt.bfloat16
Exp = mybir.ActivationFunctionType.Exp

KVE = 2 * HD         # 128 kv dims computed per core (2 kv heads)
AG_K = P * T         # 262144 elems of kT shard
AG_V = 512 * 512     # 262144 elems of v shard
AG_N = AG_K + AG_V


def _qblocks(j):
    return [j, 7 - j, j + 8, 15 - j]


def _build(repeat=1, collective=True):
    nc = bacc.Bacc("TRN2", target_bir_lowering=False, debug=False, num_devices=8)

    xtq = nc.dram_tensor("xtq", [D, OWN], bf16, kind="ExternalInput").ap()
    xtv = nc.dram_tensor("xtv", [D, 512], f32, kind="ExternalInput").ap()
    wqt = nc.dram_tensor("wqt", [D, H * HD], bf16, kind="ExternalInput").ap()
    wkt = nc.dram_tensor("wkt", [D, KV * HD], f32, kind="ExternalInput").ap()
    costv8 = nc.dram_tensor("costv8", [512, 512], f32, kind="ExternalInput").ap()
    sintv8 = nc.dram_tensor("sintv8", [512, 512], f32, kind="ExternalInput").ap()
    ident = nc.dram_tensor("ident", [P, P], f32, kind="ExternalInput").ap()
    wvt = nc.dram_tensor("wvt", [D, KV * HD], f32, kind="ExternalInput").ap()
    wot = nc.dram_tensor("wot", [H * HD, D], bf16, kind="ExternalInput").ap()
    costq = nc.dram_tensor("costq", [P, OWN], f32, kind="ExternalInput").ap()
    sintq = nc.dram_tensor("sintq", [P, OWN], f32, kind="ExternalInput").ap()
    masku = nc.dram_tensor("masku", [NB, P, P], bf16, kind="ExternalInput").ap()
    negi = nc.dram_tensor("negi", [P, P], bf16, kind="ExternalInput").ap()
    onesr = nc.dram_tensor("onesr", [1, HD], f32, kind="ExternalInput").ap()
    onesc = nc.dram_tensor("onesc", [P, NB], f32, kind="ExternalInput").ap()
    y = nc.dram_tensor("y", [OWN, D], f32, kind="ExternalOutput").ap()

    def rope_full(dst, src, cosr, sinr, t0, t1, tmp_pool, n):
        """dst[:, t0:t1] = src*cos + swap32(src)*sin_signed over 128 rows.

        cosr rows r = cos[r%64]; sinr rows are sign-folded (-sin for
        (r%64)<32, +sin otherwise). The 32-row half-swaps run on the scalar
        engine (idle during projections); the three full-width elementwise
        ops run on DVE at full lane utilization."""
        xr = tmp_pool.tile([P, 512], f32, tag="xrot", bufs=3, name="xr")
        for po in (0, 64):
            nc.scalar.copy(xr[po:po + 32, :n], src[po + 32:po + 64, :n])
            nc.scalar.copy(xr[po + 32:po + 64, :n], src[po:po + 32, :n])
        u = tmp_pool.tile([P, 512], f32, tag="ropeu", bufs=3, name="u")
        v = tmp_pool.tile([P, 512], f32, tag="ropev", bufs=3, name="v")
        nc.vector.tensor_mul(u[:, :n], src[:, :n], cosr)
        nc.vector.tensor_mul(v[:, :n], xr[:, :n], sinr)
        nc.vector.tensor_add(dst[:, t0:t1], u[:, :n], v[:, :n])

    with tile.TileContext(nc) as tc:
        _dpool_cm = tc.tile_pool(name="dram", bufs=1, space="DRAM")
        dpool = _dpool_cm.__enter__()
        _pers_cm = tc.tile_pool(name="pers", bufs=1)
        pers = _pers_cm.__enter__()

        agin = dpool.tile([AG_N], f32, tag="agin", name="agin")
        agout = dpool.tile([4, AG_N], f32, tag="agout", name="agout")
        obuf = dpool.tile([H * HD, OWN], bf16, tag="obuf", name="obuf")

        # persistent across stages: q cos/sin, qTr, ones
        cosq_t = pers.tile([P, OWN], f32, tag="cosq_t", name="cosq_t")
        sinq_t = pers.tile([P, OWN], f32, tag="sinq_t", name="sinq_t")
        ones_t = pers.tile([1, HD], f32r, tag="ones_t", name="ones_t")
        nc.sync.dma_start(cosq_t[:], costq[:])
        nc.sync.dma_start(sinq_t[:], sintq[:])
        nc.sync.dma_start(ones_t[:], onesr[:].bitcast(f32r))
        qtr = [pers.tile([P, OWN], f32r, tag=f"qtr{i}", name=f"qtr{i}")
               for i in range(16)]

        # ================= stages A-D: projections + AllGather launch ======
        def emit_body():
         with tc.tile_pool(name="consA", bufs=1) as cA, \
             tc.tile_pool(name="wk", bufs=1) as wkp, \
             tc.tile_pool(name="wst", bufs=3) as wst, \
             tc.tile_pool(name="xs", bufs=3) as xsp, \
             tc.tile_pool(name="tmp", bufs=4) as tmpp, \
             tc.tile_pool(name="cpy", bufs=3) as cpyp, \
             tc.tile_pool(name="pproj", bufs=1, space="PSUM") as pproj:


            # ---- stages A+B: token-sharded K and V projections ----
            # rope tables for K in (t, d) layout, head-tiled 8x (per-core rows)
            costd = cA.tile([P, 4 * 512], f32, tag="costd", name="costd")
            sintd = cA.tile([P, 4 * 512], f32, tag="sintd", name="sintd")
            idtA = cA.tile([P, P], f32r, tag="idtA", name="idtA")
            nc.sync.dma_start(idtA[:], ident[:].bitcast(f32r))
            krT_sh = [cA.tile([P, 512], f32, tag=f"krT{db}", name=f"krT{db}")
                      for db in range(4)]
            for vb in range(4):
                nc.sync.dma_start(costd[:, 512 * vb:512 * vb + 512],
                                  costv8[P * vb:P * vb + P, :])
                nc.sync.dma_start(sintd[:, 512 * vb:512 * vb + 512],
                                  sintv8[P * vb:P * vb + P, :])

            psk = [pproj.tile([P, 512], f32, tag=f"k{i}", bufs=1,
                              name=f"psk{i}") for i in range(4)]
            psv = [pproj.tile([P, 512], f32, tag=f"v{i}", bufs=1,
                              name=f"psv{i}") for i in range(4)]
            for cb in range(16):
                wk_t = wst.tile([P, KV * HD], f32r, tag="wk", name="wk")
                nc.sync.dma_start(wk_t[:], wkt[P * cb:P * cb + P, :].bitcast(f32r))
                wv_t = wst.tile([P, KV * HD], f32r, tag="wv", name="wv")
                nc.sync.dma_start(wv_t[:], wvt[P * cb:P * cb + P, :].bitcast(f32r))
                xv_t = xsp.tile([P, 512], f32r, tag="xtv", name="xtv")
                nc.sync.dma_start(
                    xv_t[:], xtv[P * cb:P * cb + P, :].bitcast(f32r))
                for vb in range(4):
                    nc.tensor.matmul(psk[vb][:],
                                     lhsT=xv_t[:, P * vb:P * vb + P],
                                     rhs=wk_t[:],
                                     start=(cb == 0), stop=(cb == 15))
                    nc.tensor.matmul(psv[vb][:],
                                     lhsT=xv_t[:, P * vb:P * vb + P],
                                     rhs=wv_t[:],
                                     start=(cb == 0), stop=(cb == 15))
            # K rope in (t, d) layout; write shard to agin as (512 t, 512 d)
            for vb in range(4):
                cs = costd[:, 512 * vb:512 * vb + 512]
                sn = sintd[:, 512 * vb:512 * vb + 512]
                u = tmpp.tile([P, 512], f32, tag="ropeu", bufs=3, name="u")
                nc.vector.tensor_mul(u[:], psk[vb][:], cs)
                vv = tmpp.tile([P, 512], f32, tag="ropev", bufs=3, name="vv")
                pr = psk[vb].rearrange("p (h s w) -> p h s w", s=2, w=32)
                vr = vv.rearrange("p (h s w) -> p h s w", s=2, w=32)
                sr = sn.rearrange("p (h s w) -> p h s w", s=2, w=32)
                nc.vector.tensor_mul(vr[:, :, 0, :], pr[:, :, 1, :],
                                     sr[:, :, 0, :])
                nc.vector.tensor_mul(vr[:, :, 1, :], pr[:, :, 0, :],
                                     sr[:, :, 1, :])
                kr = cpyp.tile([P, 512], f32r, tag="kr", name="kr")
                nc.vector.tensor_add(kr[:], u[:], vv[:])
                for db in range(4):
                    ptr = pproj.tile([P, P], f32r, tag=f"k{db}", bufs=1,
                                     name="ptrA")
                    nc.tensor.transpose(ptr[:], kr[:, P * db:P * db + P],
                                        idtA[:])
                    if db % 2 == 0:
                        nc.vector.tensor_copy(
                            krT_sh[db][:, P * vb:P * vb + P], ptr[:])
                    else:
                        nc.scalar.copy(
                            krT_sh[db][:, P * vb:P * vb + P], ptr[:])
            for db in range(4):
                nc.sync.dma_start(
                    agin[db * P * 512:(db + 1) * P * 512].rearrange(
                        "(p t) -> p t", t=512), krT_sh[db][:])
            for vb in range(4):
                vs = cpyp.tile([P, 512], f32, tag="vsh", name="vsh")
                nc.scalar.copy(vs[:], psv[vb][:])
                nc.sync.dma_start(
                    agin[AG_K + vb * P * 512:
                         AG_K + (vb + 1) * P * 512].rearrange(
                             "(p t) -> p t", t=512), vs[:])

            # ---- stage C: fused AllGather of (kT shard | v shard) ----
            if collective:
                nc.gpsimd.collective_compute(
                    "AllGather",
                    mybir.AluOpType.bypass,
                    replica_groups=[[0, 1, 2, 3], [4, 5, 6, 7]],
                    ins=[agin.opt()],
                    outs=[agout.opt()],
                )
            else:
                for g in range(4):
                    nc.sync.dma_start(
                        agout[g].rearrange("(a b) -> a b", b=8192),
                        agin.rearrange("(a b) -> a b", b=8192))

            # ---- stage D: Q projection (owned tokens) + rope ----
            xtq_sb = []
            for cb in range(16):
                wt = wkp.tile([P, OWN], bf16, tag=f"xq{cb}", name=f"xq{cb}")
                nc.sync.dma_start(wt[:], xtq[P * cb:P * cb + P, :])
                xtq_sb.append(wt)
            for qg in range(4):
                tg = "k" if qg % 2 == 0 else "v"
                psq = [pproj.tile([P, 512], f32, tag=f"{tg}{i}", bufs=1,
                                  name=f"psq{i}") for i in range(4)]
                for cb in range(16):
                    wq_t = wst.tile([P, 512], bf16, tag="wq", name="wq")
                    nc.sync.dma_start(
                        wq_t[:], wqt[P * cb:P * cb + P,
                                     512 * qg:512 * qg + 512])
                    for qi in range(4):
                        nc.tensor.matmul(
                            psq[qi][:], lhsT=wq_t[:, P * qi:P * qi + P],
                            rhs=xtq_sb[cb], start=(cb == 0), stop=(cb == 15))
                for qi in range(4):
                    rope_full(qtr[4 * qg + qi], psq[qi], cosq_t[:], sinq_t[:],
                              0, OWN, tmpp, OWN)

         # ================= stages E-F: unpack AG + attention ================
         with tc.tile_pool(name="persF", bufs=1) as pF, \
             tc.tile_pool(name="psS", bufs=3, space="PSUM") as psS, \
             tc.tile_pool(name="psO", bufs=2, space="PSUM") as psO, \
             tc.tile_pool(name="ptp", bufs=3) as ptp, \
             tc.tile_pool(name="nrm", bufs=2) as nrm, \
             tc.tile_pool(name="wos", bufs=1) as wos, \
             tc.tile_pool(name="otrg", bufs=1) as otrp, \
             tc.tile_pool(name="psG", bufs=1, space="PSUM") as psG, \
             tc.tile_pool(name="yc", bufs=2) as ycp:

            mask_t = pF.tile([P, NB * P], bf16, tag="mask_t", name="mask_t")
            for kb in range(NB):
                nc.sync.dma_start(mask_t[:, P * kb:P * kb + P], masku[kb])
            negi_t = pF.tile([P, P], bf16, tag="negi_t", name="negi_t")
            nc.sync.dma_start(negi_t[:], negi[:])
            ktr = [pF.tile([P, T], f32r, tag=f"ktr{g}", name=f"ktr{g}")
                   for g in range(4)]
            vaug = [pF.tile([P, NB * (HD + 1)], f32r, tag=f"vaug{kh}",
                            name=f"vaug{kh}") for kh in range(KV)]
            for g in range(4):
                ksrc = agout[g, 0:AG_K].rearrange("(d t) -> d t", t=512)
                for db in range(4):
                    nc.sync.dma_start(
                        ktr[db][:, 512 * g:512 * g + 512],
                        ksrc[P * db:P * db + P, :].bitcast(f32r))
            for kb in range(NB):
                g, lr = kb // 4, (kb % 4) * P
                vsrc = agout[g, AG_K:AG_N].rearrange("(t v) -> t v", v=512)
                vw = ptp.tile([P, 512], f32r, tag="vw", bufs=3, name="vw")
                nc.sync.dma_start(vw[:], vsrc[lr:lr + P, :].bitcast(f32r))
                for kh in range(KV):
                    if kh % 2 == 0:
                        nc.vector.tensor_copy(
                            vaug[kh][:, (HD + 1) * kb:(HD + 1) * kb + HD],
                            vw[:, HD * kh:HD * kh + HD])
                    else:
                        nc.scalar.copy(
                            vaug[kh][:, (HD + 1) * kb:(HD + 1) * kb + HD],
                            vw[:, HD * kh:HD * kh + HD])
            for kh in range(KV):
                ocol = vaug[kh].rearrange("p (k c) -> p k c", c=HD + 1)[:, :, HD]
                nc.sync.dma_start(ocol, onesc[:].bitcast(f32r))

            otr = [None] * 16
            for h in range(H):
                kh = h // 4
                kt, kpo = ktr[kh // 2], HD * (kh % 2)
                # q heads are host-permuted: tile u holds head 8*(u//4)+u%4
                # at rows 0:64 (kv even) and that head +4 at rows 64:128.
                u = (kh // 2) * 4 + h % 4
                qt, qpo = qtr[u], HD * (kh % 2)
                oslot = 2 * u + (kh % 2)
                oaug = psO.tile([P, OWN], f32, tag="oaug", name="oaug")
                for kb in range(NB):
                    qs = P * (kb // 4)
                    n = OWN - qs
                    st = psS.tile([P, OWN], f32, tag="st", bufs=6, name="st")
                    nc.tensor.matmul(
                        st[:, 0:n],
                        lhsT=kt[kpo:kpo + HD, P * kb:P * kb + P],
                        rhs=qt[qpo:qpo + HD, qs:OWN],
                        start=True, stop=True)
                    nc.tensor.matmul(st[:, 0:P], lhsT=negi_t[:],
                                     rhs=mask_t[:, P * kb:P * kb + P],
                                     start=False, stop=True,
                                     skip_group_check=True)
                    pt = ptp.tile([P, OWN], f32r, tag="pt", bufs=6, name="pt")
                    nc.scalar.activation(pt[:, 0:n], st[:, 0:n], Exp, scale=0.125)
                    nc.tensor.matmul(
                        oaug[0:HD + 1, qs:OWN],
                        lhsT=vaug[kh][:, (HD + 1) * kb:(HD + 1) * (kb + 1)],
                        rhs=pt[:, 0:n],
                        start=(kb == 0), stop=(kb == 15))
                rec = nrm.tile([1, OWN], f32, tag="rec", name="rec")
                nc.vector.reciprocal(rec[:], oaug[HD:HD + 1, :])
                recr = nrm.tile([1, OWN], f32r, tag="recr", name="recr")
                nc.vector.tensor_copy(recr[:], rec[:])
                pb = psO.tile([HD, OWN], f32, tag="oaug", name="pb")
                nc.tensor.matmul(pb[:], lhsT=ones_t[:], rhs=recr[:],
                                 start=True, stop=True)
                pbs = nrm.tile([HD, OWN], f32, tag="pbs", bufs=2, name="pbs")
                nc.vector.tensor_copy(pbs[:], pb[:])
                otmp = nrm.tile([HD, OWN], bf16, tag="otmp", bufs=3,
                                name="otmp")
                nc.vector.tensor_mul(otmp[:], oaug[0:HD, :], pbs[:])
                nc.sync.dma_start(obuf[HD * oslot:HD * oslot + HD, :], otmp[:])
                if kh % 2 == 1:
                    wt = otrp.tile([P, OWN], bf16, tag=f"otr{u}",
                                   name=f"otr{u}")
                    nc.sync.dma_start(wt[:], obuf[P * u:P * u + P, :])
                    otr[u] = wt

            # ---- stage G: o_proj (wo chunks prefetch during attention) ----
            wo_ch = {}
            for eb in range(4):
                for ab in range(16):
                    wt = wos.tile([P, 512], bf16, tag=f"wo{ab}",
                                  name=f"wo{eb}_{ab}", bufs=2)
                    nc.sync.dma_start(
                        wt[:], wot[P * ab:P * ab + P,
                                   512 * eb:512 * eb + 512])
                    wo_ch[(eb, ab)] = wt
            for eb in range(4):
                for tb in range(4):
                    # reuse the AV-accumulator banks (free once heads finish)
                    # so o_proj double-buffers without widening the budget
                    psg = psO.tile([P, 512], f32, tag="oaug", name="psg")
                    for ab in range(16):
                        nc.tensor.matmul(
                            psg[:], lhsT=otr[ab][:, P * tb:P * tb + P],
                            rhs=wo_ch[(eb, ab)][:],
                            start=(ab == 0), stop=(ab == 15))
                    yt = ycp.tile([P, 512], f32, tag="yt", name="yt")
                    nc.vector.tensor_copy(yt[:], psg[:])
                    nc.sync.dma_start(
                        y[P * tb:P * tb + P, 512 * eb:512 * eb + 512], yt[:])

        for _rep in range(repeat):
            emit_body()

        _pers_cm.__exit__(None, None, None)
        _dpool_cm.__exit__(None, None, None)

    nc.compile()
    return nc


_NC = None


def _get_nc():
    global _NC
    if _NC is None:
        _NC = _build()
    return _NC


def _head_perm():
    """Pair each even-kv head with its odd-kv partner (+4) in one 128-dim
    block, so q partition parity matches the kv head parity in kT tiles."""
    order = []
    for u in range(16):
        a = 8 * (u // 4) + u % 4
        for h in (a, a + 4):
            order.extend(range(HD * h, HD * h + HD))
    return np.asarray(order)


def _in_maps(x, cos, sin, Wq, Wk, Wv, Wo):
    xT = np.ascontiguousarray(np.transpose(np.asarray(x, np.float32), (0, 2, 1)))
    perm = _head_perm()
    import ml_dtypes as _mld
    WqT = np.ascontiguousarray(
        np.asarray(Wq, np.float32).T[:, perm].astype(_mld.bfloat16))
    WkT = np.ascontiguousarray(np.asarray(Wk, np.float32).T)
    WvT = np.ascontiguousarray(np.asarray(Wv, np.float32).T)
    import ml_dtypes
    WoT = np.ascontiguousarray(
        np.asarray(Wo, np.float32).T[perm, :].astype(ml_dtypes.bfloat16))
    cosT = np.asarray(cos, np.float32).T        # (64, T)
    sinT = np.asarray(sin, np.float32).T
    # 128-row rope tables: row r uses hd-dim r%64; sin rows sign-folded
    # (-sin for (r%64)<32) so rope is x*cos2 + swap32(x)*sin2 on 128 rows.
    sgn = np.where(np.arange(HD) < HD // 2, -1.0, 1.0).astype(np.float32)
    sgnc = sgn  # same sign-fold along the free axis for (t, d) layout
    cos2 = np.ascontiguousarray(np.tile(cosT, (2, 1)))              # (128, T)
    sin2 = np.ascontiguousarray(np.tile(sinT * sgn[:, None], (2, 1)))
    ones = np.ones((1, HD), np.float32)
    maps = []
    for c in range(8):
        b, j = c // 4, c % 4
        qb = _qblocks(j)
        cols = np.concatenate([np.arange(P * g, P * g + P) for g in qb])
        import ml_dtypes
        mask = np.empty((NB, P, P), ml_dtypes.bfloat16)
        ki = np.arange(P)[:, None]
        qi = np.arange(P)[None, :]
        for kb in range(NB):
            qg = qb[kb // 4]
            mask[kb] = np.where(P * kb + ki <= P * qg + qi, 0.0, 1.0)
        negi_np = (np.eye(P, dtype=np.float32) *
                   np.float32(-2.0 ** 30)).astype(ml_dtypes.bfloat16)
        maps.append({
            "xtq": np.ascontiguousarray(
                xT[b][:, cols].astype(_mld.bfloat16)),
            "xtv": np.ascontiguousarray(xT[b][:, 512 * j:512 * j + 512]),
            "wqt": WqT,
            "wkt": WkT,
            "wvt": WvT,
            "wot": WoT,
            "costq": np.ascontiguousarray(cos2[:, cols]),
            "sintq": np.ascontiguousarray(sin2[:, cols]),
            "costv8": np.ascontiguousarray(
                np.tile(cosT.T[512 * j:512 * j + 512, :], (1, KV))),
            "sintv8": np.ascontiguousarray(
                np.tile(sinT.T[512 * j:512 * j + 512, :] * sgnc[None, :],
                        (1, KV))),
            "ident": np.eye(P, dtype=np.float32),
            "masku": mask,
            "negi": negi_np,
            "onesr": ones,
            "onesc": np.ones((P, NB), np.float32),
        })
    return maps


def kernel(x, cos, sin, Wq, Wk, Wv, Wo):
    nc = _get_nc()
    maps = _in_maps(x, cos, sin, Wq, Wk, Wv, Wo)
    res = run_bass_kernel_spmd(nc, maps, list(range(8)))
    out = np.empty((B, T, D), np.float32)
    for c in range(8):
        b, j = c // 4, c % 4
        yc = res.results[c]["y"]
        for s, qg in enumerate(_qblocks(j)):
            out[b, P * qg:P * qg + P, :] = yc[P * s:P * s + P, :]
    return out



# revision 4
# speedup vs baseline: 1219.3252x; 1219.3252x over previous
"""Bass/Trainium2 SPMD kernel for GQA causal attention with RoPE — v2.

Sharding (8 cores): core c = 4*b + j (b = batch, j = 0..3 shard in batch).
  - K and V: token-sharded (core j computes tokens [512j, 512j+512)); one
    fused bf16 AllGather per 4-core group assembles full K^T and V.
  - Q / attention / o_proj: token-sharded at 64-token granularity: core j
    owns q 64-blocks u(p) = 4p + (j if p even else 3-j), p = 0..7 (balanced
    causal work; per-kb key-extent start(kb) = 64*(kb//2) is core-uniform).
  - Attention in s^T = (kpos, q) layout, all matmul operands bf16 (full-rate
    on PE at any width; fp32r is 1/4-rate below 256 output cols). Causal
    mask is a 64-col bf16 matmul per key block; exp on ACT merges each
    kb-pair into one instruction (two PSUM banks, one strided AP).
  - The PE is in-order, so emission order is the schedule: Q-projection
    groups are interleaved into the previous group's attention as fillers
    that hide ACT exp latency; group-0 Q-proj hides the K-transpose /
    AllGather tail of stage A.
  - o_proj reads attention outputs staged directly in SBUF; no collective
    after o_proj is needed since each core keeps all heads for its tokens.
"""
import numpy as np

import concourse.bass as bass
import concourse.tile as tile
from concourse import bacc, mybir
from concourse.bass_utils import run_bass_kernel_spmd

B, T, D = 2, 2048, 2048
H, KV, HD = 32, 8, 64
P = 128
NB = T // P          # 16 key blocks of 128
OWN = 512            # owned q tokens per core (8 x 64-blocks)
f32 = mybir.dt.float32
f32r = mybir.dt.float32r
bf16 = mybir.dt.bfloat16
Exp = mybir.ActivationFunctionType.Exp

AG_K = 4 * P * 512   # 262144 elems: kT shard, 4 d-blocks x (128 d, 512 t)
AG_V = 512 * 512     # 262144 elems: v shard (512 t, 512 v)
AG_N = AG_K + AG_V
VST = NB * (HD + 1)  # 1040: per-kv-head vaug stride (65 cols per key block)


def _q64blocks(j):
    return [4 * p + (j if p % 2 == 0 else 3 - j) for p in range(8)]


def _build(repeat=1, collective=True, upto='G'):
    nc = bacc.Bacc("TRN2", target_bir_lowering=False, debug=False, num_devices=8)

    xtq = nc.dram_tensor("xtq", [D, OWN], bf16, kind="ExternalInput").ap()
    xtv = nc.dram_tensor("xtv", [D, 512], bf16, kind="ExternalInput").ap()
    wqt = nc.dram_tensor("wqt", [D, H * HD], bf16, kind="ExternalInput").ap()
    wkt = nc.dram_tensor("wkt", [D, KV * HD], bf16, kind="ExternalInput").ap()
    wvt = nc.dram_tensor("wvt", [D, KV * HD], bf16, kind="ExternalInput").ap()
    wot = nc.dram_tensor("wot", [H * HD, D], bf16, kind="ExternalInput").ap()
    costv8 = nc.dram_tensor("costv8", [512, 512], bf16, kind="ExternalInput").ap()
    sintv8 = nc.dram_tensor("sintv8", [512, 512], bf16, kind="ExternalInput").ap()
    costq = nc.dram_tensor("costq", [P, OWN], bf16, kind="ExternalInput").ap()
    sintq = nc.dram_tensor("sintq", [P, OWN], bf16, kind="ExternalInput").ap()
    ident = nc.dram_tensor("ident", [P, P], bf16, kind="ExternalInput").ap()
    masku = nc.dram_tensor("masku", [P, NB * 64], bf16, kind="ExternalInput").ap()
    negi = nc.dram_tensor("negi", [P, P], bf16, kind="ExternalInput").ap()
    onesb = nc.dram_tensor("onesb", [1, HD], bf16, kind="ExternalInput").ap()
    onesv = nc.dram_tensor("onesv", [P, KV * NB], bf16, kind="ExternalInput").ap()
    y = nc.dram_tensor("y", [OWN, D], f32, kind="ExternalOutput").ap()

    with tile.TileContext(nc) as tc:
        _dpool_cm = tc.tile_pool(name="dram", bufs=1, space="DRAM")
        dpool = _dpool_cm.__enter__()
        _pers_cm = tc.tile_pool(name="pers", bufs=1)
        pers = _pers_cm.__enter__()

        agin = dpool.tile([AG_N], bf16, tag="agin", name="agin")
        agout = dpool.tile([4, AG_N], bf16, tag="agout", name="agout")

        def emit_body():
         with tc.tile_pool(name="wqp", bufs=2) as wqp, \
             tc.tile_pool(name="tmp", bufs=2) as tmpp, \
             tc.tile_pool(name="pt", bufs=3) as ptp, \
             tc.tile_pool(name="nrm", bufs=2) as nrm:

            # ---- persistent attention tiles ----
            xtq_t = pers.tile([P, NB * 512], bf16, tag="xtq", name="xtq_t")
            cosq_t = pers.tile([P, OWN], bf16, tag="cosq_t", name="cosq_t")
            sinq_t = pers.tile([P, OWN], bf16, tag="sinq_t", name="sinq_t")
            mask_t = pers.tile([P, NB * 64], bf16, tag="mask_t", name="mask_t")
            negi_t = pers.tile([P, P], bf16, tag="negi_t", name="negi_t")
            ones_t = pers.tile([1, HD], bf16, tag="ones_t", name="ones_t")
            onesv_t = pers.tile([P, KV * NB], bf16, tag="onesv_t",
                                name="onesv_t")
            qtr = [pers.tile([P, OWN], bf16, tag=f"qtr{i}", name=f"qtr{i}")
                   for i in range(16)]
            wq_g = [None] * 4

            def load_wq(g, eng=nc.sync, chunks=1):
                wt = wqp.tile([P, NB * 512], bf16, tag="wq", name=f"wq{g}")
                wr = wqt[:, 512 * g:512 * g + 512].rearrange(
                    "(cb p) q -> p cb q", p=P)
                w3 = wt.rearrange("p (cb q) -> p cb q", q=512)
                step = NB // chunks
                for lo in range(0, NB, step):
                    eng.dma_start(w3[:, lo:lo + step], wr[:, lo:lo + step])
                wq_g[g] = wt

            def rope_q(dst, psq):
                """dst = q*cos + swap32(q)*sin_signed, bf16 out.
                One DVE copy drains psq (so its PSUM bank frees fast); the
                half-swaps run as ACT copies, the muls/add on DVE."""
                qs = tmpp.tile([P, OWN], bf16, tag="qsb", bufs=2, name="qsb")
                nc.vector.tensor_copy(qs[:], psq[:])
                qsw = tmpp.tile([P, OWN], bf16, tag="qsw", bufs=2, name="qsw")
                for po in (0, 64):
                    nc.scalar.copy(qsw[po:po + 32, :], qs[po + 32:po + 64, :])
                    nc.scalar.copy(qsw[po + 32:po + 64, :], qs[po:po + 32, :])
                u = tmpp.tile([P, OWN], bf16, tag="qru", bufs=2, name="qru")
                nc.vector.tensor_mul(u[:], qs[:], cosq_t[:])
                v = tmpp.tile([P, OWN], bf16, tag="qrv", bufs=2, name="qrv")
                nc.vector.tensor_mul(v[:], qsw[:], sinq_t[:])
                nc.vector.tensor_add(dst[:], u[:], v[:])

            # ================== stage A + Q-proj group 0 ==================
            _psA_cm = tc.tile_pool(name="psA", bufs=1, space="PSUM")
            psA = _psA_cm.__enter__()
            with tc.tile_pool(name="pA", bufs=1) as pA:

                # input DMAs, ordered by first use
                wk_t = pA.tile([P, NB * 512], bf16, tag="wk_t", name="wk_t")
                wv_t = pA.tile([P, NB * 512], bf16, tag="wv_t", name="wv_t")
                xv_t = pA.tile([P, NB * 512], bf16, tag="xv_t", name="xv_t")
                wk3 = wk_t.rearrange("p (cb k) -> p cb k", k=512)
                wv3 = wv_t.rearrange("p (cb k) -> p cb k", k=512)
                xv3 = xv_t.rearrange("p (cb t) -> p cb t", t=512)
                wkr = wkt.rearrange("(cb p) k -> p cb k", p=P)
                wvr = wvt.rearrange("(cb p) k -> p cb k", p=P)
                xvr = xtv.rearrange("(cb p) t -> p cb t", p=P)
                nc.sync.dma_start(wk3[:, 0:2], wkr[:, 0:2])
                nc.sync.dma_start(xv3[:, 0:2], xvr[:, 0:2])
                nc.sync.dma_start(wk3[:, 2:8], wkr[:, 2:8])
                nc.sync.dma_start(xv3[:, 2:8], xvr[:, 2:8])
                nc.sync.dma_start(wv3[:, 0:8], wvr[:, 0:8])
                nc.sync.dma_start(wk3[:, 8:16], wkr[:, 8:16])
                nc.sync.dma_start(xv3[:, 8:16], xvr[:, 8:16])
                nc.sync.dma_start(wv3[:, 8:16], wvr[:, 8:16])
                costd = pA.tile([P, 4 * 512], bf16, tag="costd", name="costd")
                sintd = pA.tile([P, 4 * 512], bf16, tag="sintd", name="sintd")
                for vb in range(4):
                    nc.sync.dma_start(costd[:, 512 * vb:512 * vb + 512],
                                      costv8[P * vb:P * vb + P, :])
                    nc.sync.dma_start(sintd[:, 512 * vb:512 * vb + 512],
                                      sintv8[P * vb:P * vb + P, :])
                nc.sync.dma_start(
                    xtq_t.rearrange("p (cb t) -> p cb t", t=512),
                    xtq.rearrange("(cb p) t -> p cb t", p=P))
                load_wq(0)
                nc.sync.dma_start(cosq_t[:], costq[:])
                nc.sync.dma_start(sinq_t[:], sintq[:])
                idt = pA.tile([P, P], bf16, tag="idt", name="idt")
                nc.sync.dma_start(idt[:], ident[:])
                nc.sync.dma_start(mask_t[:], masku[:])
                nc.sync.dma_start(negi_t[:], negi[:])
                nc.sync.dma_start(ones_t[:], onesb[:])
                nc.sync.dma_start(onesv_t[:], onesv[:])
                load_wq(1)

                psk = [psA.tile([P, 512], f32, tag=f"k{i}", name=f"psk{i}")
                       for i in range(4)]
                psv = [psA.tile([P, 512], f32, tag=f"v{i}", name=f"psv{i}")
                       for i in range(4)]
                agK = agin[0:AG_K].rearrange("(db d t) -> db d t", db=4, d=P)

                def qproj_g0_steps(qi):
                    psq = psA.tile([P, 512], f32, tag=f"v{qi}",
                                   name=f"psq0_{qi}")
                    for cb in range(NB):
                        yield lambda qi=qi, cb=cb, psq=psq: nc.tensor.matmul(
                            psq[:],
                            lhsT=wq_g[0][:, 512 * cb + P * qi:
                                         512 * cb + P * qi + P],
                            rhs=xtq_t[:, 512 * cb:512 * cb + 512],
                            start=(cb == 0), stop=(cb == NB - 1))
                    yield lambda qi=qi, psq=psq: rope_q(qtr[qi], psq)

                kr = [None] * 4
                for vb in range(4):
                    for cb in range(NB):
                        nc.tensor.matmul(
                            psk[vb][:],
                            lhsT=xv_t[:, 512 * cb + P * vb:
                                      512 * cb + P * vb + P],
                            rhs=wk_t[:, 512 * cb:512 * cb + 512],
                            start=(cb == 0), stop=(cb == NB - 1))
                    # K rope in (t, d) layout: ACT swaps + 3 DVE ops
                    cs = costd[:, 512 * vb:512 * vb + 512]
                    sn = sintd[:, 512 * vb:512 * vb + 512]
                    ksw = tmpp.tile([P, 512], bf16, tag="ksw", bufs=2,
                                    name="ksw")
                    pr = psk[vb].rearrange("p (h s w) -> p h s w", s=2, w=32)
                    kwr = ksw.rearrange("p (h s w) -> p h s w", s=2, w=32)
                    nc.scalar.copy(kwr[:, :, 0, :], pr[:, :, 1, :])
                    nc.scalar.copy(kwr[:, :, 1, :], pr[:, :, 0, :])
                    u = tmpp.tile([P, 512], bf16, tag="ropeu", bufs=1,
                                  name="u")
                    nc.vector.tensor_mul(u[:], psk[vb][:], cs)
                    vv = tmpp.tile([P, 512], bf16, tag="ropev", bufs=1,
                                   name="vv")
                    nc.vector.tensor_mul(vv[:], ksw[:], sn)
                    kr[vb] = tmpp.tile([P, 512], bf16, tag=f"kr{vb}",
                                       bufs=1, name=f"kr{vb}")
                    nc.vector.tensor_add(kr[vb][:], u[:], vv[:])
                for vb in range(4):
                    # V projection for this token block + send to agin
                    for cb in range(NB):
                        nc.tensor.matmul(
                            psv[vb][:],
                            lhsT=xv_t[:, 512 * cb + P * vb:
                                      512 * cb + P * vb + P],
                            rhs=wv_t[:, 512 * cb:512 * cb + 512],
                            start=(cb == 0), stop=(cb == NB - 1))
                    vs = tmpp.tile([P, 512], bf16, tag="vsh", bufs=1,
                                   name=f"vsh{vb}")
                    nc.scalar.copy(vs[:], psv[vb][:])
                    nc.sync.dma_start(
                        agin[AG_K + vb * P * 512:
                             AG_K + (vb + 1) * P * 512].rearrange(
                                 "(p t) -> p t", t=512), vs[:])
                    # transpose kr -> krTv (PE, bank k{vb}), fill WAR stalls
                    krTv = tmpp.tile([P, 512], bf16, tag="krTv", bufs=2,
                                     name=f"krTv{vb}")
                    for db in range(4):
                        ptr = psA.tile([P, P], bf16, tag=f"k{db}", bufs=1,
                                       name=f"ptr{vb}_{db}")
                        nc.tensor.transpose(ptr[:], kr[vb][:, P * db:P * db + P],
                                            idt[:])
                        if db % 2 == 0:
                            nc.vector.tensor_copy(
                                krTv[:, P * db:P * db + P], ptr[:])
                        else:
                            nc.scalar.copy(
                                krTv[:, P * db:P * db + P], ptr[:])
                    nc.sync.dma_start(
                        agK[:, :, P * vb:P * vb + P].rearrange(
                            "db d t -> d db t"),
                        krTv.rearrange("d (db t) -> d db t", t=P))
                for qi in range(4):
                    for th in qproj_g0_steps(qi):
                        th()
                load_wq(2, chunks=4)

            # ===== post-stage-A scope: reuses stage-A SBUF for K/V/O tiles ====
            _po_cm = tc.tile_pool(name="po", bufs=1)
            po = _po_cm.__enter__()
            ktr = [po.tile([P, T], bf16, tag=f"ktr{db}", name=f"ktr{db}")
                   for db in range(4)]
            vaug = po.tile([P, KV * VST], bf16, tag="vaug", name="vaug")
            otr = [po.tile([P, OWN], bf16, tag=f"otr{i}", name=f"otr{i}")
                   for i in range(16)]

            # ================== stage B: AllGather (kT | v) ==================
            if collective:
                nc.gpsimd.collective_compute(
                    "AllGather",
                    mybir.AluOpType.bypass,
                    replica_groups=[[0, 1, 2, 3], [4, 5, 6, 7]],
                    ins=[agin.opt()],
                    outs=[agout.opt()],
                )
            else:
                for g in range(4):
                    nc.sync.dma_start(
                        agout[g].rearrange("(a b) -> a b", b=8192),
                        agin.rearrange("(a b) -> a b", b=8192))

            # ============ stage C: unpack K^T and V incrementally ============
            va4 = vaug.rearrange("p (kh kb c) -> p kh kb c", kh=KV, c=HD + 1)
            nc.vector.tensor_copy(
                va4[:, :, :, HD].rearrange("p kh kb -> p (kh kb)"), onesv_t[:])
            agKo = agout[:, 0:AG_K].rearrange("g (db d t) -> g db d t",
                                              db=4, d=P)

            def unpack_g(g):
                for db in range(4):
                    nc.sync.dma_start(
                        ktr[db][:, 512 * g:512 * g + 512], agKo[g, db])
                vt = tmpp.tile([P, 4 * 512], bf16, tag="vt", bufs=2,
                               name=f"vt{g}")
                vsrc = agout[g, AG_K:AG_N].rearrange(
                    "(kb p c) -> p kb c", kb=4, p=P)
                nc.sync.dma_start(
                    vt.rearrange("p (kb c) -> p kb c", c=512), vsrc)
                vt3 = vt.rearrange("p (kb kh c) -> p kb kh c", kb=4, kh=KV)
                for kh in range(KV):
                    nc.vector.tensor_copy(va4[:, kh, 4 * g:4 * g + 4, 0:HD],
                                          vt3[:, :, kh, :])

            unpack_g(0)

            # ========= stage D: all Q-proj as one straight PE block =========
            load_wq(3, chunks=4)
            for g in (range(1, 4) if upto >= 'D' else []):
                unpack_g(g)
                for qi in range(4):
                    psq = psA.tile([P, 512], f32, tag=f"v{qi}",
                                   bufs=1, name=f"psq{g}_{qi}")
                    for cb in range(NB):
                        nc.tensor.matmul(
                            psq[:],
                            lhsT=wq_g[g][:, 512 * cb + P * qi:
                                         512 * cb + P * qi + P],
                            rhs=xtq_t[:, 512 * cb:512 * cb + 512],
                            start=(cb == 0), stop=(cb == NB - 1))
                    rope_q(qtr[4 * g + qi], psq)
            _psA_cm.__exit__(None, None, None)

            # ========= stage E: attention, ACT-paced, depth-3 pipeline =======
            with tc.tile_pool(name="psS", bufs=1, space="PSUM") as psS, \
                 tc.tile_pool(name="psO", bufs=2, space="PSUM") as psO:

                with tc.tile_pool(name="wos", bufs=2) as wos:
                    def load_wo(eb):
                        wt = wos.tile([P, NB * 512], bf16, tag="wo",
                                      name=f"wo{eb}")
                        wr = wot[:, 512 * eb:512 * eb + 512].rearrange(
                            "(ab p) c -> p ab c", p=P)
                        w3 = wt.rearrange("p (ab c) -> p ab c", c=512)
                        for lo in range(0, NB, 4):
                            nc.sync.dma_start(w3[:, lo:lo + 4],
                                              wr[:, lo:lo + 4])
                        return wt

                    wo_nxt = None

                    prev = []
                    for h in (range(H) if upto >= 'E' else []):
                        if h == 8:
                            wo_nxt = load_wo(0)
                        kh = h // 4
                        kt, kpo = ktr[kh // 2], HD * (kh % 2)
                        uu = (kh // 2) * 4 + h % 4
                        qt, qpo = qtr[uu], HD * (kh % 2)
                        oaug = psO.tile([HD + 1, OWN], f32, tag="oa",
                                        name=f"oa{h}")
                        for t in range(8):
                            w = 512 - 64 * t
                            qs = 64 * t
                            st = psS.tile([P, 1024], f32, tag="st", bufs=3,
                                          name=f"st{h}_{t}")
                            pt = ptp.tile([P, 1024], bf16, tag="pt",
                                          name=f"pt{h}_{t}")
                            for r in range(2):
                                kb = 2 * t + r
                                nc.tensor.matmul(
                                    st[:, 512 * r:512 * r + w],
                                    lhsT=kt[kpo:kpo + HD, P * kb:P * kb + P],
                                    rhs=qt[qpo:qpo + HD, qs:OWN],
                                    start=True, stop=False)
                                nc.tensor.matmul(
                                    st[:, 512 * r:512 * r + 64],
                                    lhsT=negi_t[:],
                                    rhs=mask_t[:, 64 * kb:64 * kb + 64],
                                    start=False, stop=True,
                                    skip_group_check=True)
                            # AV trails the exp by two kb-pair steps so the
                            # in-order PE never waits on ACT latency
                            while len(prev) >= 4:
                                prev.pop(0)()
                            stv = st.rearrange("p (r c) -> p r c", c=512)
                            ptv = pt.rearrange("p (r c) -> p r c", c=512)
                            nc.scalar.activation(ptv[:, :, 0:w], stv[:, :, 0:w],
                                                 Exp, scale=0.125)
                            for r in range(2):
                                kb = 2 * t + r
                                prev.append(
                                    lambda kb=kb, w=w, qs=qs, r=r,
                                    oaug=oaug, pt=pt, kh=kh:
                                    nc.tensor.matmul(
                                        oaug[:, qs:OWN],
                                        lhsT=vaug[:, VST * kh + (HD + 1) * kb:
                                                  VST * kh + (HD + 1) * kb
                                                  + HD + 1],
                                        rhs=pt[:, 512 * r:512 * r + w],
                                        start=(kb == 0), stop=(kb == NB - 1)))
                        for th in prev:
                            th()
                        prev = []
                        rec = nrm.tile([1, OWN], bf16, tag="rec", name="rec")
                        with nc.allow_low_precision(
                                reason="softmax scale; bf16 ulp ~0.4% ok"):
                            nc.vector.reciprocal(rec[:], oaug[HD:HD + 1, :])
                        pb = psO.tile([HD, OWN], f32, tag="oa", name=f"pb{h}")
                        nc.tensor.matmul(pb[:], lhsT=ones_t[:], rhs=rec[:],
                                         start=True, stop=True)
                        pbs = nrm.tile([HD, OWN], bf16, tag="pbs", bufs=2,
                                       name=f"pbs{h}")
                        nc.vector.tensor_copy(pbs[:], pb[:])
                        nc.vector.tensor_mul(
                            otr[uu][HD * (kh % 2):HD * (kh % 2) + HD, :],
                            oaug[0:HD, :], pbs[:])

                    # ================= stage G: o_proj =================
                    if upto < 'G':
                        for tb in range(4):
                            yt = nrm.tile([P, D], f32, tag="ytd", bufs=1,
                                          name=f"ytd{tb}")
                            nc.vector.tensor_copy(yt[:], xtq_t[:, 0:D])
                            nc.sync.dma_start(y[P * tb:P * tb + P, :], yt[:])
                    for eb in (range(4) if upto >= 'G' else []):
                        wo_cur = wo_nxt
                        if eb < 3:
                            wo_nxt = load_wo(eb + 1)
                        for tb in range(4):
                            psg = psO.tile([P, 512], f32, tag="oa",
                                           name=f"pg{tb}_{eb}")
                            for ab in range(16):
                                nc.tensor.matmul(
                                    psg[:],
                                    lhsT=otr[ab][:, P * tb:P * tb + P],
                                    rhs=wo_cur[:, 512 * ab:512 * ab + 512],
                                    start=(ab == 0), stop=(ab == 15))
                            yt = nrm.tile([P, 512], f32, tag="yt", bufs=2,
                                          name=f"yt{tb}_{eb}")
                            nc.vector.tensor_copy(yt[:], psg[:])
                            nc.sync.dma_start(
                                y[P * tb:P * tb + P,
                                  512 * eb:512 * eb + 512], yt[:])
            _po_cm.__exit__(None, None, None)

        for _rep in range(repeat):
            emit_body()

        _pers_cm.__exit__(None, None, None)
        _dpool_cm.__exit__(None, None, None)

    nc.compile()
    return nc


_NC = None


def _get_nc():
    global _NC
    if _NC is None:
        _NC = _build()
    return _NC


def _head_perm():
    """Pair each even-kv head with its odd-kv partner (+4) in one 128-dim
    block, so q partition parity matches the kv head parity in kT tiles."""
    order = []
    for u in range(16):
        a = 8 * (u // 4) + u % 4
        for h in (a, a + 4):
            order.extend(range(HD * h, HD * h + HD))
    return np.asarray(order)


def _in_maps(x, cos, sin, Wq, Wk, Wv, Wo):
    import ml_dtypes
    bfl = ml_dtypes.bfloat16
    xT = np.ascontiguousarray(np.transpose(np.asarray(x, np.float32), (0, 2, 1)))
    perm = _head_perm()
    WqT = np.ascontiguousarray(np.asarray(Wq, np.float32).T[:, perm].astype(bfl))
    WkT = np.ascontiguousarray(np.asarray(Wk, np.float32).T.astype(bfl))
    WvT = np.ascontiguousarray(np.asarray(Wv, np.float32).T.astype(bfl))
    WoT = np.ascontiguousarray(np.asarray(Wo, np.float32).T[perm, :].astype(bfl))
    cosT = np.asarray(cos, np.float32).T        # (64, T)
    sinT = np.asarray(sin, np.float32).T
    # 128-row rope tables: row r uses hd-dim r%64; sin rows sign-folded
    # (-sin for (r%64)<32) so rope is x*cos2 + swap32(x)*sin2 on 128 rows.
    sgn = np.where(np.arange(HD) < HD // 2, -1.0, 1.0).astype(np.float32)
    cos2 = np.ascontiguousarray(np.tile(cosT, (2, 1)))              # (128, T)
    sin2 = np.ascontiguousarray(np.tile(sinT * sgn[:, None], (2, 1)))
    negi_np = (np.eye(P, dtype=np.float32) * np.float32(-2.0 ** 30)).astype(bfl)
    ident_np = np.eye(P, dtype=np.float32).astype(bfl)
    maps = []
    for c in range(8):
        b, j = c // 4, c % 4
        qb = _q64blocks(j)
        cols = np.concatenate([np.arange(64 * u, 64 * u + 64) for u in qb])
        mask = np.empty((P, NB * 64), bfl)
        ki = np.arange(P)[:, None]
        qi = np.arange(64)[None, :]
        for kb in range(NB):
            u = qb[kb // 2]
            mask[:, 64 * kb:64 * kb + 64] = np.where(
                P * kb + ki <= 64 * u + qi, 0.0, 1.0)
        maps.append({
            "xtq": np.ascontiguousarray(xT[b][:, cols].astype(bfl)),
            "xtv": np.ascontiguousarray(
                xT[b][:, 512 * j:512 * j + 512].astype(bfl)),
            "wqt": WqT,
            "wkt": WkT,
            "wvt": WvT,
            "wot": WoT,
            "costq": np.ascontiguousarray(cos2[:, cols].astype(bfl)),
            "sintq": np.ascontiguousarray(sin2[:, cols].astype(bfl)),
            "costv8": np.ascontiguousarray(
                np.tile(cosT.T[512 * j:512 * j + 512, :], (1, KV)).astype(bfl)),
            "sintv8": np.ascontiguousarray(
                np.tile(sinT.T[512 * j:512 * j + 512, :] * sgn[None, :],
                        (1, KV)).astype(bfl)),
            "ident": ident_np,
            "masku": mask,
            "negi": negi_np,
            "onesb": np.ones((1, HD), bfl),
            "onesv": np.ones((P, KV * NB), bfl),
        })
    return maps


def kernel(x, cos, sin, Wq, Wk, Wv, Wo):
    nc = _get_nc()
    maps = _in_maps(x, cos, sin, Wq, Wk, Wv, Wo)
    res = run_bass_kernel_spmd(nc, maps, list(range(8)))
    out = np.empty((B, T, D), np.float32)
    for c in range(8):
        b, j = c // 4, c % 4
        yc = res.results[c]["y"]
        for p, u in enumerate(_q64blocks(j)):
            out[b, 64 * u:64 * u + 64, :] = yc[64 * p:64 * p + 64, :]
    return out


# revision 6
# speedup vs baseline: 1273.4000x; 1.0443x over previous
"""Bass/Trainium2 SPMD kernel for GQA causal attention with RoPE — v2.

Sharding (8 cores): core c = 4*b + j (b = batch, j = 0..3 shard in batch).
  - K and V: token-sharded (core j computes tokens [512j, 512j+512)); one
    fused bf16 AllGather per 4-core group assembles full K^T and V.
  - Q / attention / o_proj: token-sharded at 64-token granularity: core j
    owns q 64-blocks u(p) = 4p + (j if p even else 3-j), p = 0..7 (balanced
    causal work; per-kb key-extent start(kb) = 64*(kb//2) is core-uniform).
  - Attention in s^T = (kpos, q) layout, all matmul operands bf16 (full-rate
    on PE at any width; fp32r is 1/4-rate below 256 output cols). Causal
    mask is a 64-col bf16 matmul per key block; exp on ACT merges each
    kb-pair into one instruction (two PSUM banks, one strided AP).
  - The PE is in-order, so emission order is the schedule: Q-projection
    groups are interleaved into the previous group's attention as fillers
    that hide ACT exp latency; group-0 Q-proj hides the K-transpose /
    AllGather tail of stage A.
  - o_proj reads attention outputs staged directly in SBUF; no collective
    after o_proj is needed since each core keeps all heads for its tokens.
"""
import numpy as np

import concourse.bass as bass
import concourse.tile as tile
from concourse import bacc, mybir
from concourse.bass_utils import run_bass_kernel_spmd

B, T, D = 2, 2048, 2048
H, KV, HD = 32, 8, 64
P = 128
NB = T // P          # 16 key blocks of 128
OWN = 512            # owned q tokens per core (8 x 64-blocks)
f32 = mybir.dt.float32
f32r = mybir.dt.float32r
bf16 = mybir.dt.bfloat16
Exp = mybir.ActivationFunctionType.Exp

AG_K = 4 * P * 512   # 262144 elems: kT shard, 4 d-blocks x (128 d, 512 t)
AG_V = 512 * 512     # 262144 elems: v shard (512 t, 512 v)
AG_N = AG_K + AG_V
VST = NB * (HD + 1)  # 1040: per-kv-head vaug stride (65 cols per key block)


def _q64blocks(j):
    return [4 * p + (j if p % 2 == 0 else 3 - j) for p in range(8)]


def _build(repeat=1, collective=True, upto='G'):
    nc = bacc.Bacc("TRN2", target_bir_lowering=False, debug=False, num_devices=8)

    xtq = nc.dram_tensor("xtq", [D, OWN], bf16, kind="ExternalInput").ap()
    xtv = nc.dram_tensor("xtv", [D, 512], bf16, kind="ExternalInput").ap()
    wqt = nc.dram_tensor("wqt", [D, H * HD], bf16, kind="ExternalInput").ap()
    wkt = nc.dram_tensor("wkt", [D, KV * HD], bf16, kind="ExternalInput").ap()
    wvt = nc.dram_tensor("wvt", [D, KV * HD], bf16, kind="ExternalInput").ap()
    wot = nc.dram_tensor("wot", [H * HD, D], bf16, kind="ExternalInput").ap()
    costv8 = nc.dram_tensor("costv8", [512, 512], bf16, kind="ExternalInput").ap()
    sintv8 = nc.dram_tensor("sintv8", [512, 512], bf16, kind="ExternalInput").ap()
    costq = nc.dram_tensor("costq", [P, OWN], bf16, kind="ExternalInput").ap()
    sintq = nc.dram_tensor("sintq", [P, OWN], bf16, kind="ExternalInput").ap()
    ident = nc.dram_tensor("ident", [P, P], bf16, kind="ExternalInput").ap()
    masku = nc.dram_tensor("masku", [P, NB * 64], bf16, kind="ExternalInput").ap()
    negi = nc.dram_tensor("negi", [P, P], bf16, kind="ExternalInput").ap()
    onesb = nc.dram_tensor("onesb", [1, HD], bf16, kind="ExternalInput").ap()
    onesv = nc.dram_tensor("onesv", [P, KV * NB], bf16, kind="ExternalInput").ap()
    y = nc.dram_tensor("y", [OWN, D], f32, kind="ExternalOutput").ap()

    with tile.TileContext(nc) as tc:
        _dpool_cm = tc.tile_pool(name="dram", bufs=1, space="DRAM")
        dpool = _dpool_cm.__enter__()
        _pers_cm = tc.tile_pool(name="pers", bufs=1)
        pers = _pers_cm.__enter__()

        agin = dpool.tile([AG_N], bf16, tag="agin", name="agin")
        agout = dpool.tile([4, AG_N], bf16, tag="agout", name="agout")

        def emit_body():
         with tc.tile_pool(name="wqp", bufs=2) as wqp, \
             tc.tile_pool(name="tmp", bufs=2) as tmpp, \
             tc.tile_pool(name="pt", bufs=4) as ptp, \
             tc.tile_pool(name="nrm", bufs=2) as nrm:

            # ---- persistent attention tiles ----
            xtq_t = pers.tile([P, NB * 512], bf16, tag="xtq", name="xtq_t")
            cosq_t = pers.tile([P, OWN], bf16, tag="cosq_t", name="cosq_t")
            sinq_t = pers.tile([P, OWN], bf16, tag="sinq_t", name="sinq_t")
            mask_t = pers.tile([P, NB * 64], bf16, tag="mask_t", name="mask_t")
            negi_t = pers.tile([P, P], bf16, tag="negi_t", name="negi_t")
            ones_t = pers.tile([1, HD], bf16, tag="ones_t", name="ones_t")
            onesv_t = pers.tile([P, KV * NB], bf16, tag="onesv_t",
                                name="onesv_t")
            qtr = [pers.tile([P, OWN], bf16, tag=f"qtr{i}", name=f"qtr{i}")
                   for i in range(16)]
            wq_g = [None] * 4

            def load_wq(g, eng=nc.sync, chunks=1):
                wt = wqp.tile([P, NB * 512], bf16, tag="wq", name=f"wq{g}")
                wr = wqt[:, 512 * g:512 * g + 512].rearrange(
                    "(cb p) q -> p cb q", p=P)
                w3 = wt.rearrange("p (cb q) -> p cb q", q=512)
                step = NB // chunks
                for lo in range(0, NB, step):
                    eng.dma_start(w3[:, lo:lo + step], wr[:, lo:lo + step])
                wq_g[g] = wt

            def rope_q(dst, psq):
                """dst = q*cos + swap32(q)*sin_signed, bf16 out.
                One DVE copy drains psq (so its PSUM bank frees fast); the
                half-swaps run as ACT copies, the muls/add on DVE."""
                qs = tmpp.tile([P, OWN], bf16, tag="qsb", bufs=2, name="qsb")
                nc.vector.tensor_copy(qs[:], psq[:])
                qsw = tmpp.tile([P, OWN], bf16, tag="qsw", bufs=2, name="qsw")
                for po in (0, 64):
                    nc.scalar.copy(qsw[po:po + 32, :], qs[po + 32:po + 64, :])
                    nc.scalar.copy(qsw[po + 32:po + 64, :], qs[po:po + 32, :])
                u = tmpp.tile([P, OWN], bf16, tag="qru", bufs=2, name="qru")
                nc.vector.tensor_mul(u[:], qs[:], cosq_t[:])
                v = tmpp.tile([P, OWN], bf16, tag="qrv", bufs=2, name="qrv")
                nc.vector.tensor_mul(v[:], qsw[:], sinq_t[:])
                nc.vector.tensor_add(dst[:], u[:], v[:])

            # ================== stage A + Q-proj group 0 ==================
            _psA_cm = tc.tile_pool(name="psA", bufs=1, space="PSUM")
            psA = _psA_cm.__enter__()
            with tc.tile_pool(name="pA", bufs=1) as pA:

                # input DMAs, ordered by first use
                wk_t = pA.tile([P, NB * 512], bf16, tag="wk_t", name="wk_t")
                wv_t = pA.tile([P, NB * 512], bf16, tag="wv_t", name="wv_t")
                xv_t = pA.tile([P, NB * 512], bf16, tag="xv_t", name="xv_t")
                wk3 = wk_t.rearrange("p (cb k) -> p cb k", k=512)
                wv3 = wv_t.rearrange("p (cb k) -> p cb k", k=512)
                xv3 = xv_t.rearrange("p (cb t) -> p cb t", t=512)
                wkr = wkt.rearrange("(cb p) k -> p cb k", p=P)
                wvr = wvt.rearrange("(cb p) k -> p cb k", p=P)
                xvr = xtv.rearrange("(cb p) t -> p cb t", p=P)
                nc.sync.dma_start(wk3[:, 0:1], wkr[:, 0:1])
                nc.sync.dma_start(xv3[:, 0:1], xvr[:, 0:1])
                nc.sync.dma_start(wk3[:, 1:4], wkr[:, 1:4])
                nc.sync.dma_start(xv3[:, 1:4], xvr[:, 1:4])
                nc.sync.dma_start(wk3[:, 4:8], wkr[:, 4:8])
                nc.sync.dma_start(xv3[:, 4:8], xvr[:, 4:8])
                nc.sync.dma_start(wv3[:, 0:8], wvr[:, 0:8])
                nc.sync.dma_start(wk3[:, 8:16], wkr[:, 8:16])
                nc.sync.dma_start(xv3[:, 8:16], xvr[:, 8:16])
                nc.sync.dma_start(wv3[:, 8:16], wvr[:, 8:16])
                costd = pA.tile([P, 4 * 512], bf16, tag="costd", name="costd")
                sintd = pA.tile([P, 4 * 512], bf16, tag="sintd", name="sintd")
                for vb in range(4):
                    nc.sync.dma_start(costd[:, 512 * vb:512 * vb + 512],
                                      costv8[P * vb:P * vb + P, :])
                    nc.sync.dma_start(sintd[:, 512 * vb:512 * vb + 512],
                                      sintv8[P * vb:P * vb + P, :])
                nc.sync.dma_start(
                    xtq_t.rearrange("p (cb t) -> p cb t", t=512),
                    xtq.rearrange("(cb p) t -> p cb t", p=P))
                load_wq(0)
                nc.sync.dma_start(cosq_t[:], costq[:])
                nc.sync.dma_start(sinq_t[:], sintq[:])
                idt = pA.tile([P, P], bf16, tag="idt", name="idt")
                nc.sync.dma_start(idt[:], ident[:])
                nc.sync.dma_start(mask_t[:], masku[:])
                nc.sync.dma_start(negi_t[:], negi[:])
                nc.sync.dma_start(ones_t[:], onesb[:])
                nc.sync.dma_start(onesv_t[:], onesv[:])
                load_wq(1)

                psk = [psA.tile([P, 512], f32, tag=f"k{i}", name=f"psk{i}")
                       for i in range(4)]
                psv = [psA.tile([P, 512], f32, tag=f"v{i}", name=f"psv{i}")
                       for i in range(4)]
                agK = agin[0:AG_K].rearrange("(db d t) -> db d t", db=4, d=P)

                def qproj_g0_steps(qi):
                    psq = psA.tile([P, 512], f32, tag=f"v{qi}",
                                   name=f"psq0_{qi}")
                    for cb in range(NB):
                        yield lambda qi=qi, cb=cb, psq=psq: nc.tensor.matmul(
                            psq[:],
                            lhsT=wq_g[0][:, 512 * cb + P * qi:
                                         512 * cb + P * qi + P],
                            rhs=xtq_t[:, 512 * cb:512 * cb + 512],
                            start=(cb == 0), stop=(cb == NB - 1))
                    yield lambda qi=qi, psq=psq: rope_q(qtr[qi], psq)

                kr = [None] * 4
                for vb in range(4):
                    for cb in range(NB):
                        nc.tensor.matmul(
                            psk[vb][:],
                            lhsT=xv_t[:, 512 * cb + P * vb:
                                      512 * cb + P * vb + P],
                            rhs=wk_t[:, 512 * cb:512 * cb + 512],
                            start=(cb == 0), stop=(cb == NB - 1))
                    # K rope in (t, d) layout: ACT swaps + 3 DVE ops
                    cs = costd[:, 512 * vb:512 * vb + 512]
                    sn = sintd[:, 512 * vb:512 * vb + 512]
                    ksw = tmpp.tile([P, 512], bf16, tag="ksw", bufs=2,
                                    name="ksw")
                    pr = psk[vb].rearrange("p (h s w) -> p h s w", s=2, w=32)
                    kwr = ksw.rearrange("p (h s w) -> p h s w", s=2, w=32)
                    nc.scalar.copy(kwr[:, :, 0, :], pr[:, :, 1, :])
                    nc.scalar.copy(kwr[:, :, 1, :], pr[:, :, 0, :])
                    u = tmpp.tile([P, 512], bf16, tag="ropeu", bufs=1,
                                  name="u")
                    nc.vector.tensor_mul(u[:], psk[vb][:], cs)
                    vv = tmpp.tile([P, 512], bf16, tag="ropev", bufs=1,
                                   name="vv")
                    nc.vector.tensor_mul(vv[:], ksw[:], sn)
                    kr[vb] = tmpp.tile([P, 512], bf16, tag=f"kr{vb}",
                                       bufs=1, name=f"kr{vb}")
                    nc.vector.tensor_add(kr[vb][:], u[:], vv[:])
                for vb in range(4):
                    # V projection for this token block + send to agin
                    for cb in range(NB):
                        nc.tensor.matmul(
                            psv[vb][:],
                            lhsT=xv_t[:, 512 * cb + P * vb:
                                      512 * cb + P * vb + P],
                            rhs=wv_t[:, 512 * cb:512 * cb + 512],
                            start=(cb == 0), stop=(cb == NB - 1))
                    vs = tmpp.tile([P, 512], bf16, tag="vsh", bufs=1,
                                   name=f"vsh{vb}")
                    nc.scalar.copy(vs[:], psv[vb][:])
                    nc.sync.dma_start(
                        agin[AG_K + vb * P * 512:
                             AG_K + (vb + 1) * P * 512].rearrange(
                                 "(p t) -> p t", t=512), vs[:])
                    # transpose kr -> krTv (PE, bank k{vb}), fill WAR stalls
                    krTv = tmpp.tile([P, 512], bf16, tag="krTv", bufs=2,
                                     name=f"krTv{vb}")
                    for db in range(4):
                        ptr = psA.tile([P, P], bf16, tag=f"k{db}", bufs=1,
                                       name=f"ptr{vb}_{db}")
                        nc.tensor.transpose(ptr[:], kr[vb][:, P * db:P * db + P],
                                            idt[:])
                        if db % 2 == 0:
                            nc.vector.tensor_copy(
                                krTv[:, P * db:P * db + P], ptr[:])
                        else:
                            nc.scalar.copy(
                                krTv[:, P * db:P * db + P], ptr[:])
                    nc.sync.dma_start(
                        agK[:, :, P * vb:P * vb + P].rearrange(
                            "db d t -> d db t"),
                        krTv.rearrange("d (db t) -> d db t", t=P))
                for qi in range(4):
                    for th in qproj_g0_steps(qi):
                        th()
                load_wq(2, chunks=4)

            # ===== post-stage-A scope: reuses stage-A SBUF for K/V/O tiles ====
            _po_cm = tc.tile_pool(name="po", bufs=1)
            po = _po_cm.__enter__()
            ktr = [po.tile([P, T], bf16, tag=f"ktr{db}", name=f"ktr{db}")
                   for db in range(4)]
            vaug = po.tile([P, KV * VST], bf16, tag="vaug", name="vaug")
            otr = [po.tile([P, OWN], bf16, tag=f"otr{i}", name=f"otr{i}")
                   for i in range(16)]

            # ================== stage B: AllGather (kT | v) ==================
            if collective:
                nc.gpsimd.collective_compute(
                    "AllGather",
                    mybir.AluOpType.bypass,
                    replica_groups=[[0, 1, 2, 3], [4, 5, 6, 7]],
                    ins=[agin.opt()],
                    outs=[agout.opt()],
                )
            else:
                for g in range(4):
                    nc.sync.dma_start(
                        agout[g].rearrange("(a b) -> a b", b=8192),
                        agin.rearrange("(a b) -> a b", b=8192))

            # ============ stage C: unpack K^T and V incrementally ============
            va4 = vaug.rearrange("p (kh kb c) -> p kh kb c", kh=KV, c=HD + 1)
            nc.vector.tensor_copy(
                va4[:, :, :, HD].rearrange("p kh kb -> p (kh kb)"), onesv_t[:])
            agKo = agout[:, 0:AG_K].rearrange("g (db d t) -> g db d t",
                                              db=4, d=P)

            def unpack_g(g):
                for db in range(4):
                    nc.sync.dma_start(
                        ktr[db][:, 512 * g:512 * g + 512], agKo[g, db])
                vt = tmpp.tile([P, 4 * 512], bf16, tag="vt", bufs=2,
                               name=f"vt{g}")
                vsrc = agout[g, AG_K:AG_N].rearrange(
                    "(kb p c) -> p kb c", kb=4, p=P)
                nc.sync.dma_start(
                    vt.rearrange("p (kb c) -> p kb c", c=512), vsrc)
                vt3 = vt.rearrange("p (kb kh c) -> p kb kh c", kb=4, kh=KV)
                for kh in range(KV):
                    nc.vector.tensor_copy(va4[:, kh, 4 * g:4 * g + 4, 0:HD],
                                          vt3[:, :, kh, :])

            unpack_g(0)

            # ========= stage D: all Q-proj as one straight PE block =========
            load_wq(3, chunks=4)
            for g in (range(1, 4) if upto >= 'D' else []):
                unpack_g(g)
                for qi in range(4):
                    psq = psA.tile([P, 512], f32, tag=f"v{qi}",
                                   bufs=1, name=f"psq{g}_{qi}")
                    for cb in range(NB):
                        nc.tensor.matmul(
                            psq[:],
                            lhsT=wq_g[g][:, 512 * cb + P * qi:
                                         512 * cb + P * qi + P],
                            rhs=xtq_t[:, 512 * cb:512 * cb + 512],
                            start=(cb == 0), stop=(cb == NB - 1))
                    rope_q(qtr[4 * g + qi], psq)
            _psA_cm.__exit__(None, None, None)

            # ========= stage E: attention, ACT-paced, depth-3 pipeline =======
            with tc.tile_pool(name="psS", bufs=1, space="PSUM") as psS, \
                 tc.tile_pool(name="psO", bufs=2, space="PSUM") as psO:

                with tc.tile_pool(name="wos", bufs=2) as wos:
                    def load_wo(eb):
                        wt = wos.tile([P, NB * 512], bf16, tag="wo",
                                      name=f"wo{eb}")
                        wr = wot[:, 512 * eb:512 * eb + 512].rearrange(
                            "(ab p) c -> p ab c", p=P)
                        w3 = wt.rearrange("p (ab c) -> p ab c", c=512)
                        for lo in range(0, NB, 4):
                            nc.sync.dma_start(w3[:, lo:lo + 4],
                                              wr[:, lo:lo + 4])
                        return wt

                    wo_nxt = None

                    prev = []
                    for h in (range(H) if upto >= 'E' else []):
                        if h == 8:
                            wo_nxt = load_wo(0)
                        kh = h // 4
                        kt, kpo = ktr[kh // 2], HD * (kh % 2)
                        uu = (kh // 2) * 4 + h % 4
                        qt, qpo = qtr[uu], HD * (kh % 2)
                        oaug = psO.tile([HD + 1, OWN], f32, tag="oa",
                                        name=f"oa{h}")
                        # narrow kb-pairs share one PSUM tile and one exp
                        # (slot-strided AP; the slack columns hold garbage
                        # that nothing reads), cutting ACT instructions from
                        # 8 to 6 per head.
                        for group, slotw in (((0,), 512), ((1,), 512),
                                             ((2,), 512), ((3,), 512),
                                             ((4, 5), 256), ((6, 7), 128)):
                            st = psS.tile([P, 1024], f32, tag="st", bufs=3,
                                          name=f"st{h}_{group[0]}")
                            pt = ptp.tile([P, 1024], bf16, tag="pt",
                                          name=f"pt{h}_{group[0]}")
                            for gi, t in enumerate(group):
                                w = 512 - 64 * t
                                qs = 64 * t
                                for r in range(2):
                                    kb = 2 * t + r
                                    off = slotw * (2 * gi + r)
                                    nc.tensor.matmul(
                                        st[:, off:off + w],
                                        lhsT=kt[kpo:kpo + HD,
                                                P * kb:P * kb + P],
                                        rhs=qt[qpo:qpo + HD, qs:OWN],
                                        start=True, stop=False)
                                    nc.tensor.matmul(
                                        st[:, off:off + 64],
                                        lhsT=negi_t[:],
                                        rhs=mask_t[:, 64 * kb:64 * kb + 64],
                                        start=False, stop=True,
                                        skip_group_check=True)
                            # AV trails the exp so the in-order PE never
                            # waits on ACT latency
                            while len(prev) >= 8:
                                prev.pop(0)()
                            nregions = 2 * len(group)
                            stv = st.rearrange("p (r c) -> p r c", c=slotw)
                            ptv = pt.rearrange("p (r c) -> p r c", c=slotw)
                            wmax = 512 - 64 * group[0] if len(group) == 1 \
                                else slotw
                            nc.scalar.activation(
                                ptv[:, 0:nregions, 0:wmax],
                                stv[:, 0:nregions, 0:wmax],
                                Exp, scale=0.125)
                            for gi, t in enumerate(group):
                                w = 512 - 64 * t
                                qs = 64 * t
                                for r in range(2):
                                    kb = 2 * t + r
                                    off = slotw * (2 * gi + r)
                                    prev.append(
                                        lambda kb=kb, w=w, qs=qs, off=off,
                                        oaug=oaug, pt=pt, kh=kh:
                                        nc.tensor.matmul(
                                            oaug[:, qs:OWN],
                                            lhsT=vaug[:,
                                                      VST * kh
                                                      + (HD + 1) * kb:
                                                      VST * kh
                                                      + (HD + 1) * kb
                                                      + HD + 1],
                                            rhs=pt[:, off:off + w],
                                            start=(kb == 0),
                                            stop=(kb == NB - 1)))
                        for th in prev:
                            th()
                        prev = []
                        rec = nrm.tile([1, OWN], bf16, tag="rec", name="rec")
                        with nc.allow_low_precision(
                                reason="softmax scale; bf16 ulp ~0.4% ok"):
                            nc.vector.reciprocal(rec[:], oaug[HD:HD + 1, :])
                        pb = psO.tile([HD, OWN], f32, tag="oa", name=f"pb{h}")
                        nc.tensor.matmul(pb[:], lhsT=ones_t[:], rhs=rec[:],
                                         start=True, stop=True)
                        pbs = nrm.tile([HD, OWN], bf16, tag="pbs", bufs=2,
                                       name=f"pbs{h}")
                        nc.vector.tensor_copy(pbs[:], pb[:])
                        nc.vector.tensor_mul(
                            otr[uu][HD * (kh % 2):HD * (kh % 2) + HD, :],
                            oaug[0:HD, :], pbs[:])

                    # ================= stage G: o_proj =================
                    if upto < 'G':
                        for tb in range(4):
                            yt = nrm.tile([P, D], f32, tag="ytd", bufs=1,
                                          name=f"ytd{tb}")
                            nc.vector.tensor_copy(yt[:], xtq_t[:, 0:D])
                            nc.sync.dma_start(y[P * tb:P * tb + P, :], yt[:])
                    for eb in (range(4) if upto >= 'G' else []):
                        wo_cur = wo_nxt
                        if eb < 3:
                            wo_nxt = load_wo(eb + 1)
                        for tb in range(4):
                            psg = psO.tile([P, 512], f32, tag="oa",
                                           name=f"pg{tb}_{eb}")
                            for ab in range(16):
                                nc.tensor.matmul(
                                    psg[:],
                                    lhsT=otr[ab][:, P * tb:P * tb + P],
                                    rhs=wo_cur[:, 512 * ab:512 * ab + 512],
                                    start=(ab == 0), stop=(ab == 15))
                            yt = nrm.tile([P, 512], f32, tag="yt", bufs=2,
                                          name=f"yt{tb}_{eb}")
                            nc.vector.tensor_copy(yt[:], psg[:])
                            nc.sync.dma_start(
                                y[P * tb:P * tb + P,
                                  512 * eb:512 * eb + 512], yt[:])
            _po_cm.__exit__(None, None, None)

        for _rep in range(repeat):
            emit_body()

        _pers_cm.__exit__(None, None, None)
        _dpool_cm.__exit__(None, None, None)

    nc.compile()
    return nc


_NC = None


def _get_nc():
    global _NC
    if _NC is None:
        _NC = _build()
    return _NC


def _head_perm():
    """Pair each even-kv head with its odd-kv partner (+4) in one 128-dim
    block, so q partition parity matches the kv head parity in kT tiles."""
    order = []
    for u in range(16):
        a = 8 * (u // 4) + u % 4
        for h in (a, a + 4):
            order.extend(range(HD * h, HD * h + HD))
    return np.asarray(order)


def _in_maps(x, cos, sin, Wq, Wk, Wv, Wo):
    import ml_dtypes
    bfl = ml_dtypes.bfloat16
    xT = np.ascontiguousarray(np.transpose(np.asarray(x, np.float32), (0, 2, 1)))
    perm = _head_perm()
    WqT = np.ascontiguousarray(np.asarray(Wq, np.float32).T[:, perm].astype(bfl))
    WkT = np.ascontiguousarray(np.asarray(Wk, np.float32).T.astype(bfl))
    WvT = np.ascontiguousarray(np.asarray(Wv, np.float32).T.astype(bfl))
    WoT = np.ascontiguousarray(np.asarray(Wo, np.float32).T[perm, :].astype(bfl))
    cosT = np.asarray(cos, np.float32).T        # (64, T)
    sinT = np.asarray(sin, np.float32).T
    # 128-row rope tables: row r uses hd-dim r%64; sin rows sign-folded
    # (-sin for (r%64)<32) so rope is x*cos2 + swap32(x)*sin2 on 128 rows.
    sgn = np.where(np.arange(HD) < HD // 2, -1.0, 1.0).astype(np.float32)
    cos2 = np.ascontiguousarray(np.tile(cosT, (2, 1)))              # (128, T)
    sin2 = np.ascontiguousarray(np.tile(sinT * sgn[:, None], (2, 1)))
    negi_np = (np.eye(P, dtype=np.float32) * np.float32(-2.0 ** 30)).astype(bfl)
    ident_np = np.eye(P, dtype=np.float32).astype(bfl)
    maps = []
    for c in range(8):
        b, j = c // 4, c % 4
        qb = _q64blocks(j)
        cols = np.concatenate([np.arange(64 * u, 64 * u + 64) for u in qb])
        mask = np.empty((P, NB * 64), bfl)
        ki = np.arange(P)[:, None]
        qi = np.arange(64)[None, :]
        for kb in range(NB):
            u = qb[kb // 2]
            mask[:, 64 * kb:64 * kb + 64] = np.where(
                P * kb + ki <= 64 * u + qi, 0.0, 1.0)
        maps.append({
            "xtq": np.ascontiguousarray(xT[b][:, cols].astype(bfl)),
            "xtv": np.ascontiguousarray(
                xT[b][:, 512 * j:512 * j + 512].astype(bfl)),
            "wqt": WqT,
            "wkt": WkT,
            "wvt": WvT,
            "wot": WoT,
            "costq": np.ascontiguousarray(cos2[:, cols].astype(bfl)),
            "sintq": np.ascontiguousarray(sin2[:, cols].astype(bfl)),
            "costv8": np.ascontiguousarray(
                np.tile(cosT.T[512 * j:512 * j + 512, :], (1, KV)).astype(bfl)),
            "sintv8": np.ascontiguousarray(
                np.tile(sinT.T[512 * j:512 * j + 512, :] * sgn[None, :],
                        (1, KV)).astype(bfl)),
            "ident": ident_np,
            "masku": mask,
            "negi": negi_np,
            "onesb": np.ones((1, HD), bfl),
            "onesv": np.ones((P, KV * NB), bfl),
        })
    return maps


def kernel(x, cos, sin, Wq, Wk, Wv, Wo):
    nc = _get_nc()
    maps = _in_maps(x, cos, sin, Wq, Wk, Wv, Wo)
    res = run_bass_kernel_spmd(nc, maps, list(range(8)))
    out = np.empty((B, T, D), np.float32)
    for c in range(8):
        b, j = c // 4, c % 4
        yc = res.results[c]["y"]
        for p, u in enumerate(_q64blocks(j)):
            out[b, 64 * u:64 * u + 64, :] = yc[64 * p:64 * p + 64, :]
    return out
